# revision 22
# baseline (speedup 1.0000x reference)
"""Trainium2 Bass kernel for an attention block (B=16, C=512, T=2048).

reference:
  q = wq@x + bq; k = wk@x + bk; v = wv@x + bv          (conv1x1 per sample)
  attn = softmax(q^T k over s); out = v @ attn^T
  result = gamma * out + x

Sharding: data-parallel over batch across 8 NeuronCores (2 samples/core),
weights replicated.

Device algorithm:
  - host folds gamma into wv, and gamma*bv + x into the residual xg
    (softmax rows sum to 1, so the v-bias is a per-channel constant);
    bk is dropped (a per-t constant in scores cancels in softmax over s).
    Host also pre-arranges x/xg/weights/out into the device layout
    [128p, cc, t] (original channel c == cc*128 + p) so every DMA moves
    long contiguous runs (the DMA fabric is descriptor-rate-bound).
  - q/k/scores in fp16; v/E in bf16 (exp shifted by -30, which cancels
    in softmax, to keep den in friendly fp range); PSUM always fp32.
  - head: a memset-sourced accumulation chain of warmup matmuls spins
    the PE from t~0 (opens the HAM clock gate to 8/8 early; a chain is
    not dead-code-eliminable); x streams in t-sliced on ONE queue in
    need-order (a second queue only steals fabric bandwidth).
  - phase 1: v^T[s,o] tiles via matmul(lhsT=x[c,s], rhs=(g*wv)^T[c,o]),
    interleaved with q/k per 512-t window to pace DMA arrival; q/k via
    one M=128 matmul (k rows 0:64, q rows 64:128 + bias; q DMA-shifted
    to partitions 0:64 so S^T pairs can row-pack the PE).
  - phase 2: 10 steps (7x 512-wide t-chunks + 256/128/128 at the end so
    the exposed finals chain is short). Each step runs as TWO cc-half
    passes (cc 0,1 over all 16 s-chunks, then cc 2,3), so only 4 PSUM
    banks accumulate at a time and each bank pair has half a step of
    slack for its DVE copy-out - step boundaries never stall the PE
    (the Tile scheduler orders engine queues by estimated readiness,
    so end-of-step work must never gate next-step matmuls).
  - scores/exp/den-tree for step i+1 are emitted DURING step i (and
    step 0's during phase 1): S^T into a 3-deep PSUM ring (fp16
    row-packed pairs), E = exp(S^T - 30) per half (ACT), E2 = Ea+Eb
    (GpSimd), E4/E8/E16 add-tree (DVE, bf16) -> ONE ones-matmul per
    step for den. r = 1/den via reciprocal_approx_accurate on DVE
    (ACT Ln/Exp recip thrashes activation-table loads; AF.Reciprocal
    is blocked for accuracy).
  - finals for step i run inside step i+1: result = out*r + xg -> one
    DMA out per step. Only the last (128-wide) step's finals are
    exposed (~4us); its muls split across DVE and GpSimd.
"""
import numpy as np
import ml_dtypes
import concourse.bass as bass
import concourse.bacc as bacc
import concourse.tile as tile
from concourse import mybir
from concourse.bass_utils import run_bass_kernel_spmd

F32 = mybir.dt.float32
FP16 = mybir.dt.float16
BF16 = mybir.dt.bfloat16
AF = mybir.ActivationFunctionType

B, C, T, D = 16, 512, 2048, 64
NCORES = 8
BPC = B // NCORES          # samples per core
CCH = C // 128             # 4 channel chunks
SCH = T // 128             # 16 s chunks
NPR = SCH // 2             # 8 s-chunk pairs

# phase-2 steps: (sample, t-offset, t-width)
STEPS = ([(0, tc * 512, 512) for tc in range(4)]
         + [(1, tc * 512, 512) for tc in range(3)]
         + [(1, 1536, 256), (1, 1792, 128), (1, 1920, 128)])

PROFILE = False            # set True before calling kernel() to capture HW time
LAST_EXEC_NS = None
_CACHE = {}


def _build():
    nc = bacc.Bacc("TRN2", target_bir_lowering=False, debug=False,
                   enable_asserts=False)
    xd = nc.dram_tensor("x", [BPC, 128, CCH, T], FP16,
                        kind="ExternalInput").ap()
    xgd = nc.dram_tensor("xg", [BPC, 128, CCH, T], F32,
                         kind="ExternalInput").ap()
    wkqT = nc.dram_tensor("wkqT", [128, CCH, 2 * D], FP16,
                          kind="ExternalInput").ap()
    wvT = nc.dram_tensor("wvT", [128, CCH, C], FP16,
                         kind="ExternalInput").ap()
    bqd = nc.dram_tensor("bq", [D, 1], F32, kind="ExternalInput").ap()
    onesd = nc.dram_tensor("ones", [128, 128], BF16, kind="ExternalInput").ap()
    m30d = nc.dram_tensor("m30", [128, 1], F32, kind="ExternalInput").ap()
    outd = nc.dram_tensor("out", [BPC, 128, CCH, T], F32,
                          kind="ExternalOutput").ap()

    with tile.TileContext(nc) as tc:
        with tc.tile_pool(name="const", bufs=1) as constp, \
             tc.tile_pool(name="xp", bufs=1) as xp, \
             tc.tile_pool(name="vtp", bufs=1) as vtp, \
             tc.tile_pool(name="qkp", bufs=1) as qkp, \
             tc.tile_pool(name="etp", bufs=1) as etp, \
             tc.tile_pool(name="finp", bufs=1) as finp, \
             tc.tile_pool(name="ps", bufs=1, space="PSUM") as ps:

            # ---- warmup source: memset (no DMA dependency), so the PE
            # can start spinning at t~0 while everything streams in
            warm_src = constp.tile([128, 128], BF16, name="wsrc", tag="wsrc")
            nc.gpsimd.memset(warm_src[:], 1.0)

            # ---- constants: all on the gpsimd queue (sync carries x)
            wv_big = constp.tile([128, CCH, C], FP16)
            nc.gpsimd.dma_start(out=wv_big, in_=wvT)
            wkq_big = constp.tile([128, CCH, 2 * D], FP16)
            nc.gpsimd.dma_start(out=wkq_big, in_=wkqT)
            ones = constp.tile([128, 128], BF16)
            nc.gpsimd.dma_start(out=ones, in_=onesd)
            bq_full = constp.tile([128, 1], F32)
            nc.gpsimd.dma_start(out=bq_full[D:2 * D, :], in_=bqd)
            bq_hi = bq_full[D:2 * D, :]
            m30 = constp.tile([128, 1], F32)
            nc.gpsimd.dma_start(out=m30, in_=m30d)

            # ---- x loads: sample 0 sliced on sync, sample 1 on scalar
            x_big_all = [xp.tile([128, CCH, T], FP16, name=f"x_{b}",
                                 tag=f"x{b}") for b in range(BPC)]
            for off, w in [(0, 256), (256, 256), (512, 512)]:
                nc.sync.dma_start(out=x_big_all[0][:, :, off:off + w],
                                  in_=xd[0, :, :, off:off + w])
            # the rest rides the gpsimd queue BEHIND the consts, in strict
            # need-order (sample-1 bulk last so it cannot starve sample 0)
            for off, w in [(1024, 512), (1536, 512)]:
                nc.gpsimd.dma_start(out=x_big_all[0][:, :, off:off + w],
                                    in_=xd[0, :, :, off:off + w])
            nc.gpsimd.dma_start(out=x_big_all[1], in_=xd[1])
            wv_sb = [wv_big[:, cc, :] for cc in range(CCH)]
            wkq_sb = [wkq_big[:, cc, :] for cc in range(CCH)]
            x_sb_all = [[x_big_all[b][:, cc, :] for cc in range(CCH)]
                        for b in range(BPC)]

            # ---- warmup: keep the PE busy while x streams in, so the
            # HAM clock gate opens (~3.4us of activity) before real work
            warm = ps.tile([128, 512], F32, name="warm", tag="stp", bufs=4)
            for i in range(52):
                nc.tensor.matmul(warm[:, 0:128], warm_src, warm_src,
                                 start=(i == 0), stop=(i == 51))

            # ================= phase 1: v^T and q/k, both samples ========
            vt_all, q_all, k_all = {}, {}, {}
            qhi_all, khi_all = {}, {}
            et = {}       # (si, pr) -> (e_a, e_b)
            tree = {}     # si -> dict(e2s, e4, e8, e16)

            def emit_pair(si, pr):
                """Scores + exp for (si, pr), plus the den add-tree partials
                (gpsimd e2, DVE e4/e8/e16). Emitted one step ahead of the
                consuming out-matmuls, so all of it is off the critical
                path by construction."""
                b, toff, w = STEPS[si]
                st = tree.setdefault(si, {"e2": {}, "e4": [], "e8": []})
                halves = []
                for h in range(2):
                    sc = 2 * pr + h
                    stp_h = ps.tile([128, 512], F32,
                                    name=f"st_{si}_{pr}_{h}", tag="stp",
                                    bufs=4)
                    if h == 0:
                        lhsT = k_all[b][:, sc * 128:(sc + 1) * 128]
                        rhs = q_all[b][:, toff:toff + w]
                    else:
                        lhsT = khi_all[b][D:2 * D, sc * 128:(sc + 1) * 128]
                        rhs = qhi_all[b][D:2 * D, toff:toff + w]
                    nc.tensor.matmul(stp_h[:, :w], lhsT, rhs,
                                     start=True, stop=True)
                    halves.append(stp_h)
                es = []
                for h in range(2):
                    e_h = etp.tile([128, 512], BF16,
                                   name=f"et_{si}_{pr}_{h}", tag="et",
                                   bufs=32)
                    # constant shift cancels in softmax; keeps den in a
                    # range where downstream fp stays well-behaved
                    nc.scalar.activation(out=e_h[:, :w],
                                         in_=halves[h][:, :w], func=AF.Exp,
                                         bias=m30[:])
                    es.append(e_h)
                et[(si, pr)] = tuple(es)
                e2 = etp.tile([128, 512], BF16, name=f"e2_{si}_{pr}",
                              tag="e2", bufs=3)
                nc.gpsimd.tensor_add(e2[:, :w], es[0][:, :w], es[1][:, :w])
                st["e2"][pr] = e2
                if pr % 2 == 1:
                    t4 = etp.tile([128, 512], BF16,
                                  name=f"e4_{si}_{pr // 2}", tag="e4",
                                  bufs=2)
                    nc.vector.tensor_add(t4[:, :w],
                                         st["e2"].pop(pr - 1)[:, :w],
                                         st["e2"].pop(pr)[:, :w])
                    st["e4"].append(t4)
                if pr == 3 or pr == 7:
                    t8 = etp.tile([128, 512], BF16,
                                  name=f"e8_{si}_{pr // 4}", tag="e8",
                                  bufs=2)
                    nc.vector.tensor_add(t8[:, :w], st["e4"][-2][:, :w],
                                         st["e4"][-1][:, :w])
                    st["e8"].append(t8)
                if pr == 7:
                    e16 = etp.tile([128, 512], BF16, name=f"e16_{si}",
                                   tag="e16", bufs=2)
                    nc.vector.tensor_add(e16[:, :w], st["e8"][0][:, :w],
                                         st["e8"][1][:, :w])
                    st["e16"] = e16

            for b in range(BPC):
                x_sb = x_sb_all[b]

                # v^T tiles (bf16) interleaved with q/k per 512-t
                # window, so phase-1 consumption paces the x DMA arrival
                vt_big = vtp.tile([128, SCH, C], BF16, name=f"vt_{b}",
                                  tag=f"vt{b}")
                q_hi = qkp.tile([128, T], FP16, name=f"qh_{b}", tag=f"qh{b}")
                k_hi = qkp.tile([128, T], FP16, name=f"kh_{b}", tag=f"kh{b}")
                q_sb = qkp.tile([D, T], FP16, name=f"q_{b}", tag=f"q{b}")
                k_sb = qkp.tile([D, T], FP16, name=f"k_{b}", tag=f"k{b}")
                for tc_i in range(4):
                    for sc in range(4 * tc_i, 4 * tc_i + 4):
                        vps = ps.tile([128, 512], F32, name=f"vps_{b}_{sc}",
                                      tag=f"o{'AB'[sc % 2]}0")
                        for cc in range(CCH):
                            nc.tensor.matmul(
                                vps[:], x_sb[cc][:, sc * 128:(sc + 1) * 128],
                                wv_sb[cc][:],
                                start=(cc == 0), stop=(cc == CCH - 1))
                        nc.vector.tensor_copy(out=vt_big[:, sc, :],
                                              in_=vps[:])
                        if b == 1 and sc >= 9 and sc % 2 == 1:
                            # spread step-0's scores over sample 1's tail
                            emit_pair(0, sc - 9)
                            emit_pair(0, sc - 8)
                    tsl = slice(tc_i * 512, (tc_i + 1) * 512)
                    qps = ps.tile([128, 512], F32, name=f"qps_{b}_{tc_i}",
                                  tag=f"o{'AB'[tc_i % 2]}1")
                    for cc in range(CCH):
                        nc.tensor.matmul(qps[:], wkq_sb[cc][:],
                                         x_sb[cc][:, tsl],
                                         start=(cc == 0), stop=(cc == CCH - 1))
                    nc.vector.tensor_copy(out=k_sb[:, tsl], in_=qps[0:D, :])
                    nc.scalar.activation(out=q_hi[D:2 * D, tsl],
                                         in_=qps[D:2 * D, :],
                                         func=AF.Identity, bias=bq_hi[:],
                                         scale=1.0)
                    nc.gpsimd.dma_start(out=q_sb[:, tsl],
                                        in_=q_hi[D:2 * D, tsl])
                vt_all[b] = vt_big
                nc.gpsimd.dma_start(out=k_hi[D:2 * D, :], in_=k_sb[:, :])
                q_all[b], k_all[b] = q_sb, k_sb
                qhi_all[b], khi_all[b] = q_hi, k_hi

            # ================= phase 2: attention, two cc-half passes ====
            nsteps = len(STEPS)
            pend = None   # (si, ob, xg_t) awaiting recip+finals in step si+1

            def emit_recip(si):
                b, toff, w = STEPS[si]
                den_ps = tree[si]["den"]
                scr = finp.tile([128, 512], F32, name=f"sc_{si}", tag="scr",
                                bufs=2)
                recip = finp.tile([128, 512], F32, name=f"rc_{si}", tag="rc",
                                  bufs=2)
                nc.vector.reciprocal_approx_accurate(
                    out=recip[:, :w], in_=den_ps[:, :w], scratch=scr[:, :w])
                tree[si]["recip"] = recip

            def emit_finals(si, ob, xg_t, last=False):
                b, toff, w = STEPS[si]
                if "recip" not in tree[si]:
                    emit_recip(si)
                recip = tree[si]["recip"]
                t_f = finp.tile([128, CCH, 512], F32, name=f"f_{si}",
                                tag="f", bufs=2)
                for cc in range(CCH):
                    eng = nc.gpsimd if (last and cc >= 2) else nc.vector
                    eng.tensor_mul(t_f[:, cc, :w], ob[:, cc, :w],
                                   recip[:, :w])
                    eng.tensor_add(t_f[:, cc, :w], t_f[:, cc, :w],
                                   xg_t[:, cc, :w])
                nc.sync.dma_start(out=outd[b, :, :, toff:toff + w],
                                  in_=t_f[:, :, :w])
                del tree[si]

            for si in range(nsteps):
                b, toff, w = STEPS[si]
                fut = si + 1 if si + 1 < nsteps else None
                last = si == nsteps - 1
                oacc = [ps.tile([128, 512], F32, name=f"o_{si}_{cc}",
                                tag=f"o{'AB'[cc // 2]}{cc % 2}")
                        for cc in range(CCH)]
                xg_t = finp.tile([128, CCH, 512], F32,
                                 name=f"xg_{si}", tag="xg", bufs=2)
                nc.sync.dma_start(out=xg_t[:, :, :w],
                                  in_=xgd[b, :, :, toff:toff + w])
                ob = finp.tile([128, CCH, 512], F32, name=f"ob_{si}",
                               tag="ob", bufs=2)

                def half_pass(ccs, prs_future):
                    for pr in range(NPR):
                        e_a, e_b = et[(si, pr)]
                        for h in range(2):
                            sc = 2 * pr + h
                            e_sl = (e_a if h == 0 else e_b)[:, :w]
                            for cc in ccs:
                                nc.tensor.matmul(
                                    oacc[cc][:, :w],
                                    vt_all[b][:, sc,
                                              cc * 128:(cc + 1) * 128],
                                    e_sl, start=(sc == 0),
                                    stop=(sc == SCH - 1))
                        if fut is not None and pr in (2, 6):
                            emit_pair(fut, prs_future[pr // 4 * 2])
                            emit_pair(fut, prs_future[pr // 4 * 2 + 1])
                    for cc in ccs:
                        nc.vector.tensor_copy(out=ob[:, cc, :w],
                                              in_=oacc[cc][:, :w])

                # pass A (cc 0,1): also run the previous step's finals
                half_pass((0, 1), (0, 1, 2, 3))
                if pend is not None:
                    fsi, fob, fxg = pend
                    emit_finals(fsi, fob, fxg)
                    pend = None
                # den for THIS step (its E16 completed during step si-1)
                # den borrows a ring slot: written at pass-B start, read
                # by the reciprocal ~1us later, two pairs before reuse
                den_ps = ps.tile([128, 512], F32, name=f"den_{si}",
                                 tag="stp", bufs=4)
                nc.tensor.matmul(den_ps[:, :w], ones,
                                 tree[si]["e16"][:, :w],
                                 start=True, stop=True)
                tree[si]["den"] = den_ps
                # recip right away (not with the finals): den's borrowed
                # stp ring slot frees in ~1us instead of a full step
                emit_recip(si)
                # pad the ring to a multiple of 4 allocations per step
                # (16 score tiles + den = 17), so the block<->slot phase
                # stays aligned and a block's 4th tile never chases a
                # just-issued exp from the previous block
                for pad in range(3):
                    ps.tile([128, 512], F32, name=f"pad_{si}_{pad}",
                            tag="stp", bufs=4)
                # pass B (cc 2,3)
                half_pass((2, 3), (4, 5, 6, 7))
                et_done = [et.pop((si, pr)) for pr in range(NPR)]
                del et_done
                if last:
                    emit_finals(si, ob, xg_t, last=True)
                else:
                    pend = (si, ob, xg_t)
    nc.compile()
    return nc


def _get_nc():
    if "nc" not in _CACHE:
        _CACHE["nc"] = _build()
    return _CACHE["nc"]


def kernel(x, wq, bq, wk, bk, wv, bv, gamma):
    global LAST_EXEC_NS
    g = float(np.asarray(gamma).reshape(-1)[0])
    x = np.asarray(x, np.float32)
    # fold gamma into the v path; bk cancels inside softmax; the v bias
    # contributes gamma*bv per channel (softmax rows sum to 1) -> fold it
    # plus the residual into xg
    wvT = np.ascontiguousarray(
        (g * np.asarray(wv, np.float32)).T.reshape(CCH, 128, C)
        .transpose(1, 0, 2)).astype(np.float16)
    wkqT = np.ascontiguousarray(
        np.concatenate([np.asarray(wk, np.float32).T,
                        np.asarray(wq, np.float32).T], axis=1)
        .reshape(CCH, 128, 2 * D).transpose(1, 0, 2)).astype(np.float16)
    bq2 = np.asarray(bq, np.float32).reshape(D, 1)
    gbv = (g * np.asarray(bv, np.float32)).reshape(1, C, 1)
    xg = x + gbv
    # device layout: [B, p, cc, T] with original c == cc*128 + p
    xg = np.ascontiguousarray(
        xg.reshape(B, CCH, 128, T).transpose(0, 2, 1, 3))
    ones = np.ones((128, 128), ml_dtypes.bfloat16)
    m30c = np.full((128, 1), -30.0, np.float32)
    xh = np.ascontiguousarray(
        x.reshape(B, CCH, 128, T).transpose(0, 2, 1, 3)).astype(np.float16)

    in_maps = []
    for core in range(NCORES):
        sl = slice(core * BPC, (core + 1) * BPC)
        in_maps.append({
            "x": xh[sl], "xg": xg[sl],
            "wkqT": wkqT, "wvT": wvT,
            "bq": bq2, "ones": ones, "m30": m30c,
        })

    nc = _get_nc()
    res = run_bass_kernel_spmd(nc, in_maps, core_ids=list(range(NCORES)),
                               trace=PROFILE)
    LAST_EXEC_NS = res.exec_time_ns
    out = np.empty((B, 128, CCH, T), np.float32)
    for core in range(NCORES):
        out[core * BPC:(core + 1) * BPC] = res.results[core]["out"]
    return np.ascontiguousarray(
        out.transpose(0, 2, 1, 3)).reshape(B, C, T)
